# revision 33
# baseline (speedup 1.0000x reference)
"""Multi-head attention (B=2, S=2048, D=1024, H=16) on 8 Trainium2 cores.

Sharding: (batch, head-group-of-4) -> 8 cores, Megatron-style. Core c
handles batch c//4 and heads 4*(c%4)..4*(c%4)+3 (d_local = 256 columns of
Wq/Wk/Wv, 256 rows of Wo). Each core computes a partial [2048, 1024]
output; the host sums the 4 partials per batch (row-parallel Wo).

Key-side truncation: softmax keys are masked per batch to valid_lens;
only ceil(max(valid_lens)/128) key tiles are ever computed (the rest
contribute exp(-1e6) = 0). The mask is applied as a per-partition bias
on the ScalarE exp that evacuates score PSUM tiles (scores are computed
transposed: [key, query]).

Precision: activations/weights stream in as fp16 (inputs are ~N(0,1), so
fp16's 11-bit mantissa costs ~5e-5 rel per element); all matmuls run
single-pass (fp16 or raw-fp32 "float32r", 1 cycle/row); PSUM accumulates
fp32. Partial outputs return as fp16 and are summed in fp32 on host.

The kernel program is built at call time from the actual valid_lens, so
any input values work; shapes are hardcoded to this problem.
"""
import sys
if "/opt/trn_rl_repo" not in sys.path:
    sys.path.insert(0, "/opt/trn_rl_repo")
import os
import time
import numpy as np

B, SQ, SK, D, H, HD = 2, 2048, 2048, 1024, 16, 64
NEG = -1.0e6
N_CORES = 8
DL = 256          # d_local: 4 heads * 64
KD = D // 128     # contraction tiles over D

_NC_CACHE = {}
last_results = None
last_exec_wall_s = None

# "f16": fp16 streams and fp16 attention core (fast path; FWL weight loads)
# "f32r": all-fp32 storage, single-pass raw-fp32 matmuls
# "f32": exact fp32 (4 cycles/row matmuls)
PREC = os.environ.get("BASS_MHA_PREC", "f16")


def _build(KT, prec=None):
    import concourse.bass as bass  # noqa: F401
    import concourse.tile as tile
    from concourse import bacc, mybir

    prec = PREC if prec is None else prec
    f32 = mybir.dt.float32
    f16 = mybir.dt.float16
    # matmul-operand dtype for the attention core (scores/ctx/out-proj).
    # fp16 (not float32r) so LDWEIGHTS gets the fast-weight-load path on HW.
    md = {"f32": f32, "f32r": mybir.dt.float32r, "f16": f16}[prec]
    # dtype of the streamed activations/weights (and their matmuls)
    xd = f16 if prec == "f16" else md
    # output dtype
    od = f16 if prec == "f16" else f32

    LK = KT * 128
    kchunks = [(i * 512, min(512, LK - i * 512)) for i in range((LK + 511) // 512)]

    nc = bacc.Bacc("TRN2", target_bir_lowering=False, debug=False,
                   num_devices=N_CORES)
    xqT = nc.dram_tensor("xqT", [D, SQ], xd, kind="ExternalInput")
    xkT = nc.dram_tensor("xkT", [D, LK], xd, kind="ExternalInput")
    xvT = nc.dram_tensor("xvT", [D, LK], xd, kind="ExternalInput")
    wq = nc.dram_tensor("wq", [D, DL], xd, kind="ExternalInput")
    wk = nc.dram_tensor("wk", [D, DL], xd, kind="ExternalInput")
    wv = nc.dram_tensor("wv", [D, DL], xd, kind="ExternalInput")
    wo = nc.dram_tensor("wo", [DL, D], md, kind="ExternalInput")
    mask = nc.dram_tensor("mask", [128, KT], f32, kind="ExternalInput")
    out = nc.dram_tensor("out", [SQ, D], od, kind="ExternalOutput")
    dbg = os.environ.get("BASS_MHA_DEBUG") == "1"
    if dbg:
        dbg_qt = nc.dram_tensor("dbg_qt", [128, 2, SQ], md, kind="ExternalOutput")
        dbg_kt = nc.dram_tensor("dbg_kt", [128, 2, LK], md, kind="ExternalOutput")
        dbg_v = nc.dram_tensor("dbg_v", [128, KT, 4, 128], md, kind="ExternalOutput")

    with tile.TileContext(nc) as tc:
        with tc.tile_pool(name="singles", bufs=1) as sg:
            wq_sb = sg.tile([128, KD, DL], xd)
            wk_sb = sg.tile([128, KD, DL], xd)
            wv_sb = sg.tile([128, KD, DL], xd)
            wo_sb = sg.tile([128, DL // 128, D], md)
            mask_sb = sg.tile([128, KT], f32)
            kt_sb = sg.tile([128, 2, LK], md)       # K^T  [d_local, key]
            v_sb = sg.tile([128, KT, 4, 128], md)   # V''  [key, head, d | ones]
            qt_sb = sg.tile([128, 2, SQ], md)       # Q^T  [d_local, query]
            ctxT_sb = sg.tile([128, 2, SQ], md)     # Ctx^T normalized

            # DMA issue order = arrival order: K path, Q path, V path, Wo
            nc.sync.dma_start(out=mask_sb, in_=mask[:, :])
            nc.sync.dma_start(out=wk_sb, in_=wk[:, :].rearrange("(k p) j -> p k j", p=128))
            if md == f16:
                nc.vector.memset(v_sb, 1.0)
            else:
                nc.vector.memset(v_sb.bitcast(f32), 1.0)
            # dummy exp: pulls the ~2.7us activation-table load off phase C's
            # critical path (ACT is otherwise idle until the first softmax)
            warm_sb = sg.tile([1, 1], f32)
            nc.scalar.activation(warm_sb, mask_sb[0:1, 0:1],
                                 mybir.ActivationFunctionType.Exp)

            # ---- resident input streams (DMA priority: xk, xq, xv) ----
            strm_cm = tc.tile_pool(name="streams", bufs=1)
            strm = strm_cm.__enter__()
            xk_full = strm.tile([128, KD, LK], xd)
            xq_full = strm.tile([128, KD, SQ], xd)
            xv_full = strm.tile([128, KD, LK], xd)
            for k in range(KD):
                nc.sync.dma_start(out=xk_full[:, k, :],
                                  in_=xkT[k * 128:(k + 1) * 128, :])

            # ---- Phase A1: K^T = (Wk^T blocks) @ Xk^T, [256, LK] ----
            with tc.tile_pool(name="psA", bufs=1, space="PSUM") as psA:
                accs = {}
                for m in range(2):
                    for ci, (c0, cw) in enumerate(kchunks):
                        accs[(m, ci)] = psA.tile([128, cw], f32,
                                                 tag=f"kt{m}_{ci}", name=f"kt{m}_{ci}")
                for k in range(KD):
                    for m in range(2):
                        for ci, (c0, cw) in enumerate(kchunks):
                            nc.tensor.matmul(accs[(m, ci)],
                                             wk_sb[:, k, m * 128:(m + 1) * 128],
                                             xk_full[:, k, c0:c0 + cw],
                                             start=(k == 0), stop=(k == KD - 1))
                for m in range(2):
                    for ci, (c0, cw) in enumerate(kchunks):
                        nc.vector.tensor_copy(kt_sb[:, m, c0:c0 + cw], accs[(m, ci)])

            # ---- Phases B + A2: Q^T and V projections riding the DMA ----
            # xq streams in right after xk; Q accumulates in narrow passes
            # (2 or 4 PSUM banks) so the V accumulators (1 bank per key
            # tile, bank-aligned: matmul start=True clears a whole bank)
            # fit alongside. Pass 0 rides the xq stream; V rides xv.
            nc.sync.dma_start(out=wq_sb, in_=wq[:, :].rearrange("(k p) j -> p k j", p=128))
            if KT <= 4:
                qpass, nchunk = 2, 2
            else:
                qpass, nchunk = 4, 1
            maxg = 8 - 2 * nchunk
            vgroups = [list(range(g0, min(g0 + maxg, KT)))
                       for g0 in range(0, KT, maxg)]
            with tc.tile_pool(name="psB", bufs=1, space="PSUM") as psB, \
                 tc.tile_pool(name="psV", bufs=1, space="PSUM") as psV:
                for k in range(KD):
                    nc.sync.dma_start(out=xq_full[:, k, :],
                                      in_=xqT[k * 128:(k + 1) * 128, :])
                nc.sync.dma_start(out=wv_sb, in_=wv[:, :].rearrange("(k p) j -> p k j", p=128))
                for k in range(KD):
                    nc.sync.dma_start(out=xv_full[:, k, :],
                                      in_=xvT[k * 128:(k + 1) * 128, :])
                for p in range(qpass):
                    chunks = list(range(p * nchunk, (p + 1) * nchunk))
                    qaccs = {}
                    for m in range(2):
                        for c in chunks:
                            qaccs[(m, c)] = psB.tile([128, 512], f32,
                                                     tag=f"q{m}_{c % nchunk}",
                                                     name=f"qp{p}_{m}_{c}")
                    for k in range(KD):
                        for m in range(2):
                            for c in chunks:
                                nc.tensor.matmul(
                                    qaccs[(m, c)],
                                    wq_sb[:, k, m * 128:(m + 1) * 128],
                                    xq_full[:, k, c * 512:(c + 1) * 512],
                                    start=(k == 0), stop=(k == KD - 1))
                    if p < len(vgroups):
                        g = vgroups[p]
                        vacc = psV.tile([128, len(g), 512], f32, tag="vg",
                                        name=f"vg{p}")
                        for k in range(KD):
                            for vi, t in enumerate(g):
                                nc.tensor.matmul(
                                    vacc[:, vi, 0:DL],
                                    xv_full[:, k, t * 128:(t + 1) * 128],
                                    wv_sb[:, k, :],
                                    start=(k == 0), stop=(k == KD - 1),
                                    skip_group_check=True)
                        for vi, t in enumerate(g):
                            for hh in range(4):
                                nc.vector.tensor_copy(
                                    v_sb[:, t, hh, 0:64],
                                    vacc[:, vi, hh * 64:(hh + 1) * 64])
                    for m in range(2):
                        for c in chunks:
                            nc.vector.tensor_copy(
                                qt_sb[:, m, c * 512:(c + 1) * 512], qaccs[(m, c)])

            strm_cm.__exit__(None, None, None)
            nc.sync.dma_start(out=wo_sb, in_=wo[:, :].rearrange("(k p) j -> p k j", p=128))

            # ---- Phase C: per-head attention ----
            # scores^T tile = K^T_h.T @ Q^T_h  -> exp(bias=mask) -> P^T
            # ctx'' = V''.T @ P^T : rows 0-63 ctx, rows 64-127 denominator
            with tc.tile_pool(name="pt", bufs=6) as ptp, \
                 tc.tile_pool(name="misc", bufs=4) as mp, \
                 tc.tile_pool(name="psS", bufs=2, space="PSUM") as psS, \
                 tc.tile_pool(name="psC", bufs=2, space="PSUM") as psC:
                for half in range(2):
                    for hh in range(4):
                        mt, mo = hh // 2, 64 * (hh % 2)
                        h0 = half * 1024
                        ctx_ps = psC.tile([128, 1024], f32, tag="ctx")
                        for t in range(KT):
                            pt_t = ptp.tile([128, 1024], md, tag="pt")
                            s_ps = psS.tile([128, 1024], f32, tag="s")
                            for cq in range(2):
                                nc.tensor.matmul(
                                    s_ps[:, cq * 512:(cq + 1) * 512],
                                    kt_sb[mo:mo + 64, mt, t * 128:(t + 1) * 128],
                                    qt_sb[mo:mo + 64, mt, h0 + cq * 512:h0 + (cq + 1) * 512],
                                    start=True, stop=True)
                            nc.scalar.activation(
                                pt_t, s_ps,
                                mybir.ActivationFunctionType.Exp,
                                bias=mask_sb[:, t:t + 1], scale=0.125)
                            for c in range(2):
                                nc.tensor.matmul(ctx_ps[:, c * 512:(c + 1) * 512],
                                                 v_sb[:, t, hh, :],
                                                 pt_t[:, c * 512:(c + 1) * 512],
                                                 start=(t == 0), stop=(t == KT - 1),
                                                 skip_group_check=True)
                        # rows 64-127 of ctx_ps all hold the softmax denominator
                        rcb = mp.tile([64, 1024], f32, tag="rcb")
                        nc.vector.reciprocal(rcb, ctx_ps[64:128, :])
                        nc.vector.tensor_mul(ctxT_sb[mo:mo + 64, mt, h0:h0 + 1024],
                                             ctx_ps[0:64, :], rcb)

            # ---- Phase D: partial output projection ----
            with tc.tile_pool(name="po", bufs=6) as pop, \
                 tc.tile_pool(name="psD", bufs=2, space="PSUM") as psD:
                for qi in range(SQ // 128):
                    o_ps = psD.tile([128, D], f32, tag="o")
                    for n in range(2):
                        for kk in range(2):
                            nc.tensor.matmul(o_ps[:, n * 512:(n + 1) * 512],
                                             ctxT_sb[:, kk, qi * 128:(qi + 1) * 128],
                                             wo_sb[:, kk, n * 512:(n + 1) * 512],
                                             start=(kk == 0), stop=(kk == 1))
                    o_sb = pop.tile([128, D], od, tag="o_sb")
                    nc.scalar.copy(o_sb[:, 0:512], o_ps[:, 0:512])
                    nc.vector.tensor_copy(o_sb[:, 512:1024], o_ps[:, 512:1024])
                    nc.sync.dma_start(out=out[qi * 128:(qi + 1) * 128, :], in_=o_sb)
            if dbg:
                nc.sync.dma_start(out=dbg_qt[:, :, :], in_=qt_sb)
                nc.sync.dma_start(out=dbg_kt[:, :, :], in_=kt_sb)
                nc.sync.dma_start(out=dbg_v[:, :, :, :], in_=v_sb)
    nc.compile()
    return nc


def kernel(**inputs):
    global last_results, last_exec_wall_s
    from concourse.bass_utils import run_bass_kernel_spmd

    # BASS_TRACE needs the axon NTFF hook; disable tracing when the hook
    # module is unavailable so a stray env var cannot crash the run.
    if os.environ.get("BASS_TRACE"):
        try:
            from antenv import axon_hooks  # noqa: F401
        except Exception:
            os.environ["BASS_NEVER_TRACE"] = "1"

    q = np.asarray(inputs["queries"], dtype=np.float32)
    kx = np.asarray(inputs["keys"], dtype=np.float32)
    vx = np.asarray(inputs["values"], dtype=np.float32)
    vl = np.asarray(inputs["valid_lens"], dtype=np.int64).reshape(B)
    Wq = np.asarray(inputs["Wq"], dtype=np.float32)
    Wk = np.asarray(inputs["Wk"], dtype=np.float32)
    Wv = np.asarray(inputs["Wv"], dtype=np.float32)
    Wo = np.asarray(inputs["Wo"], dtype=np.float32)
    assert q.shape == (B, SQ, D) and kx.shape == (B, SK, D) and vx.shape == (B, SK, D)

    lens = np.clip(vl, 1, SK)
    lmax = int(lens.max())
    KT = (lmax + 127) // 128
    LK = KT * 128

    key = (KT, PREC)
    if key not in _NC_CACHE:
        _NC_CACHE[key] = _build(KT)
    nc = _NC_CACHE[key]

    xdt = np.float16 if PREC == "f16" else np.float32

    in_maps = []
    for c in range(N_CORES):
        b, hg = c // 4, c % 4
        cols = slice(DL * hg, DL * (hg + 1))
        m = np.where(np.arange(LK) < lens[b], 0.0, NEG).astype(np.float32)
        in_maps.append({
            "xqT": np.ascontiguousarray(q[b].T.astype(xdt)),
            "xkT": np.ascontiguousarray(kx[b, :LK].T.astype(xdt)),
            "xvT": np.ascontiguousarray(vx[b, :LK].T.astype(xdt)),
            "wq": np.ascontiguousarray(Wq[:, cols].astype(xdt)),
            "wk": np.ascontiguousarray(Wk[:, cols].astype(xdt)),
            "wv": np.ascontiguousarray(Wv[:, cols].astype(xdt)),
            "wo": np.ascontiguousarray(Wo[cols, :].astype(xdt)),
            "mask": np.ascontiguousarray(m.reshape(KT, 128).T),
        })

    t0 = time.perf_counter()
    res = run_bass_kernel_spmd(nc, in_maps, core_ids=list(range(N_CORES)))
    last_exec_wall_s = time.perf_counter() - t0
    last_results = res

    outs = [res.results[c]["out"].astype(np.float32) for c in range(N_CORES)]
    full = np.stack([outs[0] + outs[1] + outs[2] + outs[3],
                     outs[4] + outs[5] + outs[6] + outs[7]])
    return full.astype(np.float32)


# revision 34
# speedup vs baseline: 1.0531x; 1.0531x over previous
"""Multi-head attention (B=2, S=2048, D=1024, H=16) on 8 Trainium2 cores.

Sharding: (batch, head-group-of-4) -> 8 cores, Megatron-style. Core c
handles batch c//4 and heads 4*(c%4)..4*(c%4)+3 (d_local = 256 columns of
Wq/Wk/Wv, 256 rows of Wo). Each core computes a partial [2048, 1024]
output; the host sums the 4 partials per batch (row-parallel Wo).

Key-side truncation: softmax keys are masked per batch to valid_lens;
only ceil(max(valid_lens)/128) key tiles are ever computed (the rest
contribute exp(-1e6) = 0). The mask is applied as a per-partition bias
on the ScalarE exp that evacuates score PSUM tiles (scores are computed
transposed: [key, query]).

Precision: activations/weights stream in as fp16 (inputs are ~N(0,1), so
fp16's 11-bit mantissa costs ~5e-5 rel per element); all matmuls run
single-pass (fp16 or raw-fp32 "float32r", 1 cycle/row); PSUM accumulates
fp32. Partial outputs return as fp16 and are summed in fp32 on host.

The kernel program is built at call time from the actual valid_lens, so
any input values work; shapes are hardcoded to this problem.
"""
import sys
if "/opt/trn_rl_repo" not in sys.path:
    sys.path.insert(0, "/opt/trn_rl_repo")
import os
import time
import numpy as np

B, SQ, SK, D, H, HD = 2, 2048, 2048, 1024, 16, 64
NEG = -1.0e6
N_CORES = 8
DL = 256          # d_local: 4 heads * 64
KD = D // 128     # contraction tiles over D

_NC_CACHE = {}
last_results = None
last_exec_wall_s = None

# "f16": fp16 streams and fp16 attention core (fast path; FWL weight loads)
# "f32r": all-fp32 storage, single-pass raw-fp32 matmuls
# "f32": exact fp32 (4 cycles/row matmuls)
PREC = os.environ.get("BASS_MHA_PREC", "f16")


def _build(KT, prec=None):
    import concourse.bass as bass  # noqa: F401
    import concourse.tile as tile
    from concourse import bacc, mybir

    prec = PREC if prec is None else prec
    f32 = mybir.dt.float32
    f16 = mybir.dt.float16
    # matmul-operand dtype for the attention core (scores/ctx/out-proj).
    # fp16 (not float32r) so LDWEIGHTS gets the fast-weight-load path on HW.
    md = {"f32": f32, "f32r": mybir.dt.float32r, "f16": f16}[prec]
    # dtype of the streamed activations/weights (and their matmuls)
    xd = f16 if prec == "f16" else md
    # output dtype
    od = f16 if prec == "f16" else f32

    LK = KT * 128
    kchunks = [(i * 512, min(512, LK - i * 512)) for i in range((LK + 511) // 512)]

    nc = bacc.Bacc("TRN2", target_bir_lowering=False, debug=False,
                   num_devices=N_CORES)
    xqT = nc.dram_tensor("xqT", [D, SQ], xd, kind="ExternalInput")
    xkT = nc.dram_tensor("xkT", [D, LK], xd, kind="ExternalInput")
    xvT = nc.dram_tensor("xvT", [D, LK], xd, kind="ExternalInput")
    wq = nc.dram_tensor("wq", [D, DL], xd, kind="ExternalInput")
    wk = nc.dram_tensor("wk", [D, DL], xd, kind="ExternalInput")
    wv = nc.dram_tensor("wv", [D, DL], xd, kind="ExternalInput")
    wo = nc.dram_tensor("wo", [DL, D], md, kind="ExternalInput")
    mask = nc.dram_tensor("mask", [128, KT], f32, kind="ExternalInput")
    out = nc.dram_tensor("out", [SQ, D], od, kind="ExternalOutput")
    dbg = os.environ.get("BASS_MHA_DEBUG") == "1"
    if dbg:
        dbg_qt = nc.dram_tensor("dbg_qt", [128, 2, SQ], md, kind="ExternalOutput")
        dbg_kt = nc.dram_tensor("dbg_kt", [128, 2, LK], md, kind="ExternalOutput")
        dbg_v = nc.dram_tensor("dbg_v", [128, KT, 4, 128], md, kind="ExternalOutput")

    with tile.TileContext(nc) as tc:
        with tc.tile_pool(name="singles", bufs=1) as sg:
            wq_sb = sg.tile([128, KD, DL], xd)
            wk_sb = sg.tile([128, KD, DL], xd)
            wv_sb = sg.tile([128, KD, DL], xd)
            wo_sb = sg.tile([128, DL // 128, D], md)
            mask_sb = sg.tile([128, KT], f32)
            kt_sb = sg.tile([128, 2, LK], md)       # K^T  [d_local, key]
            v_sb = sg.tile([128, KT, 4, 128], md)   # V''  [key, head, d | ones]
            qt_sb = sg.tile([128, 2, SQ], md)       # Q^T  [d_local, query]
            ctxT_sb = sg.tile([128, 2, SQ], md)     # Ctx^T normalized

            # DMA issue order = arrival order: K path, Q path, V path, Wo
            nc.sync.dma_start(out=mask_sb, in_=mask[:, :])
            nc.sync.dma_start(out=wk_sb, in_=wk[:, :].rearrange("(k p) j -> p k j", p=128))
            if md == f16:
                nc.vector.memset(v_sb, 1.0)
            else:
                nc.vector.memset(v_sb.bitcast(f32), 1.0)
            # dummy exp: pulls the ~2.7us activation-table load off phase C's
            # critical path (ACT is otherwise idle until the first softmax)
            warm_sb = sg.tile([1, 1], f32)
            nc.scalar.activation(warm_sb, mask_sb[0:1, 0:1],
                                 mybir.ActivationFunctionType.Exp)

            # ---- resident input streams (DMA priority: xk, xq, xv) ----
            strm_cm = tc.tile_pool(name="streams", bufs=1)
            strm = strm_cm.__enter__()
            xk_full = strm.tile([128, KD, LK], xd)
            xq_full = strm.tile([128, KD, SQ], xd)
            xv_full = strm.tile([128, KD, LK], xd)
            for k in range(KD):
                nc.sync.dma_start(out=xk_full[:, k, :],
                                  in_=xkT[k * 128:(k + 1) * 128, :])

            # ---- Phase A1: K^T = (Wk^T blocks) @ Xk^T, [256, LK] ----
            with tc.tile_pool(name="psA", bufs=1, space="PSUM") as psA:
                accs = {}
                for m in range(2):
                    for ci, (c0, cw) in enumerate(kchunks):
                        accs[(m, ci)] = psA.tile([128, cw], f32,
                                                 tag=f"kt{m}_{ci}", name=f"kt{m}_{ci}")
                for k in range(KD):
                    for m in range(2):
                        for ci, (c0, cw) in enumerate(kchunks):
                            nc.tensor.matmul(accs[(m, ci)],
                                             wk_sb[:, k, m * 128:(m + 1) * 128],
                                             xk_full[:, k, c0:c0 + cw],
                                             start=(k == 0), stop=(k == KD - 1))
                for m in range(2):
                    for ci, (c0, cw) in enumerate(kchunks):
                        nc.vector.tensor_copy(kt_sb[:, m, c0:c0 + cw], accs[(m, ci)])

            # ---- Phases B + A2: Q^T and V projections riding the DMA ----
            # xq streams in right after xk; Q accumulates in narrow passes
            # (2 or 4 PSUM banks) so the V accumulators (1 bank per key
            # tile, bank-aligned: matmul start=True clears a whole bank)
            # fit alongside. Pass 0 rides the xq stream; V rides xv.
            nc.sync.dma_start(out=wq_sb, in_=wq[:, :].rearrange("(k p) j -> p k j", p=128))
            if KT <= 4:
                qpass, nchunk = 2, 2
            else:
                qpass, nchunk = 4, 1
            maxg = 8 - 2 * nchunk
            vgroups = [list(range(g0, min(g0 + maxg, KT)))
                       for g0 in range(0, KT, maxg)]
            with tc.tile_pool(name="psB", bufs=1, space="PSUM") as psB, \
                 tc.tile_pool(name="psV", bufs=1, space="PSUM") as psV:
                for k in range(KD):
                    nc.sync.dma_start(out=xq_full[:, k, :],
                                      in_=xqT[k * 128:(k + 1) * 128, :])
                nc.sync.dma_start(out=wv_sb, in_=wv[:, :].rearrange("(k p) j -> p k j", p=128))
                for k in range(KD):
                    nc.sync.dma_start(out=xv_full[:, k, :],
                                      in_=xvT[k * 128:(k + 1) * 128, :])
                for p in range(qpass):
                    chunks = list(range(p * nchunk, (p + 1) * nchunk))
                    qaccs = {}
                    for m in range(2):
                        for c in chunks:
                            qaccs[(m, c)] = psB.tile([128, 512], f32,
                                                     tag=f"q{m}_{c % nchunk}",
                                                     name=f"qp{p}_{m}_{c}")
                    for k in range(KD):
                        for m in range(2):
                            for c in chunks:
                                nc.tensor.matmul(
                                    qaccs[(m, c)],
                                    wq_sb[:, k, m * 128:(m + 1) * 128],
                                    xq_full[:, k, c * 512:(c + 1) * 512],
                                    start=(k == 0), stop=(k == KD - 1))
                    if p < len(vgroups):
                        g = vgroups[p]
                        vacc = psV.tile([128, len(g), 512], f32, tag="vg",
                                        name=f"vg{p}")
                        for k in range(KD):
                            for vi, t in enumerate(g):
                                nc.tensor.matmul(
                                    vacc[:, vi, 0:DL],
                                    xv_full[:, k, t * 128:(t + 1) * 128],
                                    wv_sb[:, k, :],
                                    start=(k == 0), stop=(k == KD - 1),
                                    skip_group_check=True)
                        for vi, t in enumerate(g):
                            for hh in range(4):
                                nc.vector.tensor_copy(
                                    v_sb[:, t, hh, 0:64],
                                    vacc[:, vi, hh * 64:(hh + 1) * 64])
                    for m in range(2):
                        for c in chunks:
                            nc.vector.tensor_copy(
                                qt_sb[:, m, c * 512:(c + 1) * 512], qaccs[(m, c)])

            strm_cm.__exit__(None, None, None)
            nc.sync.dma_start(out=wo_sb, in_=wo[:, :].rearrange("(k p) j -> p k j", p=128))

            # ---- Phase C: per-head attention ----
            # scores^T tile = K^T_h.T @ Q^T_h  -> exp(bias=mask) -> P^T
            # ctx'' = V''.T @ P^T : rows 0-63 ctx, rows 64-127 denominator
            with tc.tile_pool(name="pt", bufs=6) as ptp, \
                 tc.tile_pool(name="misc", bufs=4) as mp, \
                 tc.tile_pool(name="psC", bufs=2, space="PSUM") as psC, \
                 tc.tile_pool(name="psS", bufs=2, space="PSUM") as psS:
                for half in range(2):
                    for hh in range(4):
                        mt, mo = hh // 2, 64 * (hh % 2)
                        h0 = half * 1024
                        ctx_ps = psC.tile([128, 1024], f32, tag="ctx")
                        for t in range(KT):
                            pt_t = ptp.tile([128, 1024], md, tag="pt")
                            s_ps = psS.tile([128, 1024], f32, tag="s")
                            for cq in range(2):
                                nc.tensor.matmul(
                                    s_ps[:, cq * 512:(cq + 1) * 512],
                                    kt_sb[mo:mo + 64, mt, t * 128:(t + 1) * 128],
                                    qt_sb[mo:mo + 64, mt, h0 + cq * 512:h0 + (cq + 1) * 512],
                                    start=True, stop=True)
                            nc.scalar.activation(
                                pt_t, s_ps,
                                mybir.ActivationFunctionType.Exp,
                                bias=mask_sb[:, t:t + 1], scale=0.125)
                            for c in range(2):
                                nc.tensor.matmul(ctx_ps[:, c * 512:(c + 1) * 512],
                                                 v_sb[:, t, hh, :],
                                                 pt_t[:, c * 512:(c + 1) * 512],
                                                 start=(t == 0), stop=(t == KT - 1),
                                                 skip_group_check=True)
                        # rows 64-127 of ctx_ps all hold the softmax denominator
                        rcb = mp.tile([64, 1024], f32, tag="rcb")
                        nc.vector.reciprocal(rcb, ctx_ps[64:128, :])
                        nc.vector.tensor_mul(ctxT_sb[mo:mo + 64, mt, h0:h0 + 1024],
                                             ctx_ps[0:64, :], rcb)

            # ---- Phase D: partial output projection ----
            with tc.tile_pool(name="po", bufs=6) as pop, \
                 tc.tile_pool(name="psD", bufs=4, space="PSUM") as psD:
                for qi in range(SQ // 128):
                    o_ps = psD.tile([128, D], f32, tag="o")
                    for n in range(2):
                        for kk in range(2):
                            nc.tensor.matmul(o_ps[:, n * 512:(n + 1) * 512],
                                             ctxT_sb[:, kk, qi * 128:(qi + 1) * 128],
                                             wo_sb[:, kk, n * 512:(n + 1) * 512],
                                             start=(kk == 0), stop=(kk == 1))
                    o_sb = pop.tile([128, D], od, tag="o_sb")
                    nc.scalar.copy(o_sb[:, 0:512], o_ps[:, 0:512])
                    nc.vector.tensor_copy(o_sb[:, 512:1024], o_ps[:, 512:1024])
                    nc.sync.dma_start(out=out[qi * 128:(qi + 1) * 128, :], in_=o_sb)
            if dbg:
                nc.sync.dma_start(out=dbg_qt[:, :, :], in_=qt_sb)
                nc.sync.dma_start(out=dbg_kt[:, :, :], in_=kt_sb)
                nc.sync.dma_start(out=dbg_v[:, :, :, :], in_=v_sb)
    nc.compile()
    return nc


def kernel(**inputs):
    global last_results, last_exec_wall_s
    from concourse.bass_utils import run_bass_kernel_spmd

    # BASS_TRACE needs the axon NTFF hook; disable tracing when the hook
    # module is unavailable so a stray env var cannot crash the run.
    if os.environ.get("BASS_TRACE"):
        try:
            from antenv import axon_hooks  # noqa: F401
        except Exception:
            os.environ["BASS_NEVER_TRACE"] = "1"

    q = np.asarray(inputs["queries"], dtype=np.float32)
    kx = np.asarray(inputs["keys"], dtype=np.float32)
    vx = np.asarray(inputs["values"], dtype=np.float32)
    vl = np.asarray(inputs["valid_lens"], dtype=np.int64).reshape(B)
    Wq = np.asarray(inputs["Wq"], dtype=np.float32)
    Wk = np.asarray(inputs["Wk"], dtype=np.float32)
    Wv = np.asarray(inputs["Wv"], dtype=np.float32)
    Wo = np.asarray(inputs["Wo"], dtype=np.float32)
    assert q.shape == (B, SQ, D) and kx.shape == (B, SK, D) and vx.shape == (B, SK, D)

    lens = np.clip(vl, 1, SK)
    lmax = int(lens.max())
    KT = (lmax + 127) // 128
    LK = KT * 128

    key = (KT, PREC)
    if key not in _NC_CACHE:
        _NC_CACHE[key] = _build(KT)
    nc = _NC_CACHE[key]

    xdt = np.float16 if PREC == "f16" else np.float32

    in_maps = []
    for c in range(N_CORES):
        b, hg = c // 4, c % 4
        cols = slice(DL * hg, DL * (hg + 1))
        m = np.where(np.arange(LK) < lens[b], 0.0, NEG).astype(np.float32)
        in_maps.append({
            "xqT": np.ascontiguousarray(q[b].T.astype(xdt)),
            "xkT": np.ascontiguousarray(kx[b, :LK].T.astype(xdt)),
            "xvT": np.ascontiguousarray(vx[b, :LK].T.astype(xdt)),
            "wq": np.ascontiguousarray(Wq[:, cols].astype(xdt)),
            "wk": np.ascontiguousarray(Wk[:, cols].astype(xdt)),
            "wv": np.ascontiguousarray(Wv[:, cols].astype(xdt)),
            "wo": np.ascontiguousarray(Wo[cols, :].astype(xdt)),
            "mask": np.ascontiguousarray(m.reshape(KT, 128).T),
        })

    t0 = time.perf_counter()
    res = run_bass_kernel_spmd(nc, in_maps, core_ids=list(range(N_CORES)))
    last_exec_wall_s = time.perf_counter() - t0
    last_results = res

    outs = [res.results[c]["out"].astype(np.float32) for c in range(N_CORES)]
    full = np.stack([outs[0] + outs[1] + outs[2] + outs[3],
                     outs[4] + outs[5] + outs[6] + outs[7]])
    return full.astype(np.float32)


# revision 35
# speedup vs baseline: 1.0756x; 1.0214x over previous
"""Multi-head attention (B=2, S=2048, D=1024, H=16) on 8 Trainium2 cores.

Sharding: (batch, head-group-of-4) -> 8 cores, Megatron-style. Core c
handles batch c//4 and heads 4*(c%4)..4*(c%4)+3 (d_local = 256 columns of
Wq/Wk/Wv, 256 rows of Wo). Each core computes a partial [2048, 1024]
output; the host sums the 4 partials per batch (row-parallel Wo).

Key-side truncation: softmax keys are masked per batch to valid_lens;
only ceil(max(valid_lens)/128) key tiles are ever computed (the rest
contribute exp(-1e6) = 0). The mask is applied as a per-partition bias
on the ScalarE exp that evacuates score PSUM tiles (scores are computed
transposed: [key, query]).

Precision: activations/weights stream in as fp16 (inputs are ~N(0,1), so
fp16's 11-bit mantissa costs ~5e-5 rel per element); all matmuls run
single-pass (fp16 or raw-fp32 "float32r", 1 cycle/row); PSUM accumulates
fp32. Partial outputs return as fp16 and are summed in fp32 on host.

The kernel program is built at call time from the actual valid_lens, so
any input values work; shapes are hardcoded to this problem.
"""
import sys
if "/opt/trn_rl_repo" not in sys.path:
    sys.path.insert(0, "/opt/trn_rl_repo")
import os
import time
import numpy as np

B, SQ, SK, D, H, HD = 2, 2048, 2048, 1024, 16, 64
NEG = -1.0e6
N_CORES = 8
DL = 256          # d_local: 4 heads * 64
KD = D // 128     # contraction tiles over D

_NC_CACHE = {}
last_results = None
last_exec_wall_s = None

# "f16": fp16 streams and fp16 attention core (fast path; FWL weight loads)
# "f32r": all-fp32 storage, single-pass raw-fp32 matmuls
# "f32": exact fp32 (4 cycles/row matmuls)
PREC = os.environ.get("BASS_MHA_PREC", "f16")


def _build(KT, prec=None):
    import concourse.bass as bass  # noqa: F401
    import concourse.tile as tile
    from concourse import bacc, mybir

    prec = PREC if prec is None else prec
    f32 = mybir.dt.float32
    f16 = mybir.dt.float16
    # matmul-operand dtype for the attention core (scores/ctx/out-proj).
    # fp16 (not float32r) so LDWEIGHTS gets the fast-weight-load path on HW.
    md = {"f32": f32, "f32r": mybir.dt.float32r, "f16": f16}[prec]
    # dtype of the streamed activations/weights (and their matmuls)
    xd = f16 if prec == "f16" else md
    # output dtype
    od = f16 if prec == "f16" else f32

    LK = KT * 128
    kchunks = [(i * 512, min(512, LK - i * 512)) for i in range((LK + 511) // 512)]

    nc = bacc.Bacc("TRN2", target_bir_lowering=False, debug=False,
                   num_devices=N_CORES)
    xqT = nc.dram_tensor("xqT", [D, SQ], xd, kind="ExternalInput")
    xkT = nc.dram_tensor("xkT", [D, LK], xd, kind="ExternalInput")
    xvT = nc.dram_tensor("xvT", [D, LK], xd, kind="ExternalInput")
    wq = nc.dram_tensor("wq", [D, DL], xd, kind="ExternalInput")
    wk = nc.dram_tensor("wk", [D, DL], xd, kind="ExternalInput")
    wv = nc.dram_tensor("wv", [D, DL], xd, kind="ExternalInput")
    wo = nc.dram_tensor("wo", [DL, D], md, kind="ExternalInput")
    mask = nc.dram_tensor("mask", [128, KT], f32, kind="ExternalInput")
    out = nc.dram_tensor("out", [SQ, D], od, kind="ExternalOutput")
    dbg = os.environ.get("BASS_MHA_DEBUG") == "1"
    if dbg:
        dbg_qt = nc.dram_tensor("dbg_qt", [128, 2, SQ], md, kind="ExternalOutput")
        dbg_kt = nc.dram_tensor("dbg_kt", [128, 2, LK], md, kind="ExternalOutput")
        dbg_v = nc.dram_tensor("dbg_v", [128, KT, 4, 128], md, kind="ExternalOutput")

    with tile.TileContext(nc) as tc:
        with tc.tile_pool(name="singles", bufs=1) as sg:
            wq_sb = sg.tile([128, KD, DL], xd)
            wk_sb = sg.tile([128, KD, DL], xd)
            wv_sb = sg.tile([128, KD, DL], xd)
            wo_sb = sg.tile([128, DL // 128, D], md)
            mask_sb = sg.tile([128, KT], f32)
            kt_sb = sg.tile([128, 2, LK], md)       # K^T  [d_local, key]
            v_sb = sg.tile([128, KT, 4, 128], md)   # V''  [key, head, d | ones]
            qt_sb = sg.tile([128, 2, SQ], md)       # Q^T  [d_local, query]
            ctxT_sb = sg.tile([128, 2, SQ], md)     # Ctx^T normalized

            # DMA issue order = arrival order: K path, Q path, V path, Wo
            nc.sync.dma_start(out=mask_sb, in_=mask[:, :])
            nc.sync.dma_start(out=wk_sb, in_=wk[:, :].rearrange("(k p) j -> p k j", p=128))
            if md == f16:
                nc.vector.memset(v_sb, 1.0)
            else:
                nc.vector.memset(v_sb.bitcast(f32), 1.0)
            # dummy exp: pulls the ~2.7us activation-table load off phase C's
            # critical path (ACT is otherwise idle until the first softmax)
            warm_sb = sg.tile([1, 1], f32)
            nc.scalar.activation(warm_sb, mask_sb[0:1, 0:1],
                                 mybir.ActivationFunctionType.Exp)

            # ---- resident input streams (DMA priority: xk, xq, xv) ----
            strm_cm = tc.tile_pool(name="streams", bufs=1)
            strm = strm_cm.__enter__()
            xk_full = strm.tile([128, KD, LK], xd)
            xq_full = strm.tile([128, KD, SQ], xd)
            xv_full = strm.tile([128, KD, LK], xd)
            for k in range(KD):
                nc.sync.dma_start(out=xk_full[:, k, :],
                                  in_=xkT[k * 128:(k + 1) * 128, :])

            # ---- Phase A1: K^T = (Wk^T blocks) @ Xk^T, [256, LK] ----
            with tc.tile_pool(name="psA", bufs=1, space="PSUM") as psA:
                accs = {}
                for m in range(2):
                    for ci, (c0, cw) in enumerate(kchunks):
                        accs[(m, ci)] = psA.tile([128, cw], f32,
                                                 tag=f"kt{m}_{ci}", name=f"kt{m}_{ci}")
                for k in range(KD):
                    for m in range(2):
                        for ci, (c0, cw) in enumerate(kchunks):
                            nc.tensor.matmul(accs[(m, ci)],
                                             wk_sb[:, k, m * 128:(m + 1) * 128],
                                             xk_full[:, k, c0:c0 + cw],
                                             start=(k == 0), stop=(k == KD - 1))
                for m in range(2):
                    for ci, (c0, cw) in enumerate(kchunks):
                        nc.vector.tensor_copy(kt_sb[:, m, c0:c0 + cw], accs[(m, ci)])

            # ---- Phases B + A2: Q^T and V projections riding the DMA ----
            # xq streams in right after xk; Q accumulates in narrow passes
            # (2 or 4 PSUM banks) so the V accumulators (1 bank per key
            # tile, bank-aligned: matmul start=True clears a whole bank)
            # fit alongside. Pass 0 rides the xq stream; V rides xv.
            nc.sync.dma_start(out=wq_sb, in_=wq[:, :].rearrange("(k p) j -> p k j", p=128))
            if KT <= 8:
                qpass, nchunk = 2, 2
            else:
                qpass, nchunk = 4, 1
            maxg = 8 - 2 * nchunk
            vgroups = [list(range(g0, min(g0 + maxg, KT)))
                       for g0 in range(0, KT, maxg)]
            with tc.tile_pool(name="psB", bufs=1, space="PSUM") as psB, \
                 tc.tile_pool(name="psV", bufs=1, space="PSUM") as psV:
                for k in range(KD):
                    nc.sync.dma_start(out=xq_full[:, k, :],
                                      in_=xqT[k * 128:(k + 1) * 128, :])
                nc.sync.dma_start(out=wv_sb, in_=wv[:, :].rearrange("(k p) j -> p k j", p=128))
                for k in range(KD):
                    nc.sync.dma_start(out=xv_full[:, k, :],
                                      in_=xvT[k * 128:(k + 1) * 128, :])
                for p in range(qpass):
                    chunks = list(range(p * nchunk, (p + 1) * nchunk))
                    qaccs = {}
                    for m in range(2):
                        for c in chunks:
                            qaccs[(m, c)] = psB.tile([128, 512], f32,
                                                     tag=f"q{m}_{c % nchunk}",
                                                     name=f"qp{p}_{m}_{c}")
                    for k in range(KD):
                        for m in range(2):
                            for c in chunks:
                                nc.tensor.matmul(
                                    qaccs[(m, c)],
                                    wq_sb[:, k, m * 128:(m + 1) * 128],
                                    xq_full[:, k, c * 512:(c + 1) * 512],
                                    start=(k == 0), stop=(k == KD - 1))
                    if p < len(vgroups):
                        g = vgroups[p]
                        vacc = psV.tile([128, len(g), 512], f32, tag="vg",
                                        name=f"vg{p}")
                        for k in range(KD):
                            for vi, t in enumerate(g):
                                nc.tensor.matmul(
                                    vacc[:, vi, 0:DL],
                                    xv_full[:, k, t * 128:(t + 1) * 128],
                                    wv_sb[:, k, :],
                                    start=(k == 0), stop=(k == KD - 1),
                                    skip_group_check=True)
                        for vi, t in enumerate(g):
                            for hh in range(4):
                                nc.vector.tensor_copy(
                                    v_sb[:, t, hh, 0:64],
                                    vacc[:, vi, hh * 64:(hh + 1) * 64])
                    for m in range(2):
                        for c in chunks:
                            nc.vector.tensor_copy(
                                qt_sb[:, m, c * 512:(c + 1) * 512], qaccs[(m, c)])

            strm_cm.__exit__(None, None, None)
            nc.sync.dma_start(out=wo_sb, in_=wo[:, :].rearrange("(k p) j -> p k j", p=128))

            # ---- Phase C: per-head attention ----
            # scores^T tile = K^T_h.T @ Q^T_h  -> exp(bias=mask) -> P^T
            # ctx'' = V''.T @ P^T : rows 0-63 ctx, rows 64-127 denominator
            with tc.tile_pool(name="pt", bufs=6) as ptp, \
                 tc.tile_pool(name="misc", bufs=4) as mp, \
                 tc.tile_pool(name="psC", bufs=2, space="PSUM") as psC, \
                 tc.tile_pool(name="psS", bufs=2, space="PSUM") as psS:
                for half in range(2):
                    for hh in range(4):
                        mt, mo = hh // 2, 64 * (hh % 2)
                        h0 = half * 1024
                        ctx_ps = psC.tile([128, 1024], f32, tag="ctx")
                        for t in range(KT):
                            pt_t = ptp.tile([128, 1024], md, tag="pt")
                            s_ps = psS.tile([128, 1024], f32, tag="s")
                            for cq in range(2):
                                nc.tensor.matmul(
                                    s_ps[:, cq * 512:(cq + 1) * 512],
                                    kt_sb[mo:mo + 64, mt, t * 128:(t + 1) * 128],
                                    qt_sb[mo:mo + 64, mt, h0 + cq * 512:h0 + (cq + 1) * 512],
                                    start=True, stop=True)
                            nc.scalar.activation(
                                pt_t, s_ps,
                                mybir.ActivationFunctionType.Exp,
                                bias=mask_sb[:, t:t + 1], scale=0.125)
                            for c in range(2):
                                nc.tensor.matmul(ctx_ps[:, c * 512:(c + 1) * 512],
                                                 v_sb[:, t, hh, :],
                                                 pt_t[:, c * 512:(c + 1) * 512],
                                                 start=(t == 0), stop=(t == KT - 1),
                                                 skip_group_check=True)
                        # rows 64-127 of ctx_ps all hold the softmax denominator
                        rcb = mp.tile([64, 1024], f32, tag="rcb")
                        nc.vector.reciprocal(rcb, ctx_ps[64:128, :])
                        nc.vector.tensor_mul(ctxT_sb[mo:mo + 64, mt, h0:h0 + 1024],
                                             ctx_ps[0:64, :], rcb)

            # ---- Phase D: partial output projection ----
            with tc.tile_pool(name="po", bufs=6) as pop, \
                 tc.tile_pool(name="psD", bufs=4, space="PSUM") as psD:
                for qi in range(SQ // 128):
                    o_ps = psD.tile([128, D], f32, tag="o")
                    for n in range(2):
                        for kk in range(2):
                            nc.tensor.matmul(o_ps[:, n * 512:(n + 1) * 512],
                                             ctxT_sb[:, kk, qi * 128:(qi + 1) * 128],
                                             wo_sb[:, kk, n * 512:(n + 1) * 512],
                                             start=(kk == 0), stop=(kk == 1))
                    o_sb = pop.tile([128, D], od, tag="o_sb")
                    nc.scalar.copy(o_sb[:, 0:512], o_ps[:, 0:512])
                    nc.vector.tensor_copy(o_sb[:, 512:1024], o_ps[:, 512:1024])
                    nc.sync.dma_start(out=out[qi * 128:(qi + 1) * 128, :], in_=o_sb)
            if dbg:
                nc.sync.dma_start(out=dbg_qt[:, :, :], in_=qt_sb)
                nc.sync.dma_start(out=dbg_kt[:, :, :], in_=kt_sb)
                nc.sync.dma_start(out=dbg_v[:, :, :, :], in_=v_sb)
    nc.compile()
    return nc


def kernel(**inputs):
    global last_results, last_exec_wall_s
    from concourse.bass_utils import run_bass_kernel_spmd

    # BASS_TRACE needs the axon NTFF hook; disable tracing when the hook
    # module is unavailable so a stray env var cannot crash the run.
    if os.environ.get("BASS_TRACE"):
        try:
            from antenv import axon_hooks  # noqa: F401
        except Exception:
            os.environ["BASS_NEVER_TRACE"] = "1"

    q = np.asarray(inputs["queries"], dtype=np.float32)
    kx = np.asarray(inputs["keys"], dtype=np.float32)
    vx = np.asarray(inputs["values"], dtype=np.float32)
    vl = np.asarray(inputs["valid_lens"], dtype=np.int64).reshape(B)
    Wq = np.asarray(inputs["Wq"], dtype=np.float32)
    Wk = np.asarray(inputs["Wk"], dtype=np.float32)
    Wv = np.asarray(inputs["Wv"], dtype=np.float32)
    Wo = np.asarray(inputs["Wo"], dtype=np.float32)
    assert q.shape == (B, SQ, D) and kx.shape == (B, SK, D) and vx.shape == (B, SK, D)

    lens = np.clip(vl, 1, SK)
    lmax = int(lens.max())
    KT = (lmax + 127) // 128
    LK = KT * 128

    key = (KT, PREC)
    if key not in _NC_CACHE:
        _NC_CACHE[key] = _build(KT)
    nc = _NC_CACHE[key]

    xdt = np.float16 if PREC == "f16" else np.float32

    in_maps = []
    for c in range(N_CORES):
        b, hg = c // 4, c % 4
        cols = slice(DL * hg, DL * (hg + 1))
        m = np.where(np.arange(LK) < lens[b], 0.0, NEG).astype(np.float32)
        in_maps.append({
            "xqT": np.ascontiguousarray(q[b].T.astype(xdt)),
            "xkT": np.ascontiguousarray(kx[b, :LK].T.astype(xdt)),
            "xvT": np.ascontiguousarray(vx[b, :LK].T.astype(xdt)),
            "wq": np.ascontiguousarray(Wq[:, cols].astype(xdt)),
            "wk": np.ascontiguousarray(Wk[:, cols].astype(xdt)),
            "wv": np.ascontiguousarray(Wv[:, cols].astype(xdt)),
            "wo": np.ascontiguousarray(Wo[cols, :].astype(xdt)),
            "mask": np.ascontiguousarray(m.reshape(KT, 128).T),
        })

    t0 = time.perf_counter()
    res = run_bass_kernel_spmd(nc, in_maps, core_ids=list(range(N_CORES)))
    last_exec_wall_s = time.perf_counter() - t0
    last_results = res

    outs = [res.results[c]["out"].astype(np.float32) for c in range(N_CORES)]
    full = np.stack([outs[0] + outs[1] + outs[2] + outs[3],
                     outs[4] + outs[5] + outs[6] + outs[7]])
    return full.astype(np.float32)


# revision 37
# speedup vs baseline: 1.0764x; 1.0007x over previous
"""Multi-head attention (B=2, S=2048, D=1024, H=16) on 8 Trainium2 cores.

Sharding: (batch, head-group-of-4) -> 8 cores, Megatron-style. Core c
handles batch c//4 and heads 4*(c%4)..4*(c%4)+3 (d_local = 256 columns of
Wq/Wk/Wv, 256 rows of Wo). Each core computes a partial [2048, 1024]
output; the host sums the 4 partials per batch (row-parallel Wo).

Key-side truncation: softmax keys are masked per batch to valid_lens;
only ceil(max(valid_lens)/128) key tiles are ever computed (the rest
contribute exp(-1e6) = 0). The mask is applied as a per-partition bias
on the ScalarE exp that evacuates score PSUM tiles (scores are computed
transposed: [key, query]).

Precision: activations/weights stream in as fp16 (inputs are ~N(0,1), so
fp16's 11-bit mantissa costs ~5e-5 rel per element); all matmuls run
single-pass (fp16 or raw-fp32 "float32r", 1 cycle/row); PSUM accumulates
fp32. Partial outputs return as fp16 and are summed in fp32 on host.

The kernel program is built at call time from the actual valid_lens, so
any input values work; shapes are hardcoded to this problem.
"""
import sys
if "/opt/trn_rl_repo" not in sys.path:
    sys.path.insert(0, "/opt/trn_rl_repo")
import os
import time
import numpy as np

B, SQ, SK, D, H, HD = 2, 2048, 2048, 1024, 16, 64
NEG = -1.0e6
N_CORES = 8
DL = 256          # d_local: 4 heads * 64
KD = D // 128     # contraction tiles over D

_NC_CACHE = {}
last_results = None
last_exec_wall_s = None

# "f16": fp16 streams and fp16 attention core (fast path; FWL weight loads)
# "f32r": all-fp32 storage, single-pass raw-fp32 matmuls
# "f32": exact fp32 (4 cycles/row matmuls)
PREC = os.environ.get("BASS_MHA_PREC", "f16")


def _build(KT, prec=None):
    import concourse.bass as bass  # noqa: F401
    import concourse.tile as tile
    from concourse import bacc, mybir

    prec = PREC if prec is None else prec
    f32 = mybir.dt.float32
    f16 = mybir.dt.float16
    # matmul-operand dtype for the attention core (scores/ctx/out-proj).
    # fp16 (not float32r) so LDWEIGHTS gets the fast-weight-load path on HW.
    md = {"f32": f32, "f32r": mybir.dt.float32r, "f16": f16}[prec]
    # dtype of the streamed activations/weights (and their matmuls)
    xd = f16 if prec == "f16" else md
    # output dtype
    od = f16 if prec == "f16" else f32

    LK = KT * 128
    kchunks = [(i * 512, min(512, LK - i * 512)) for i in range((LK + 511) // 512)]

    nc = bacc.Bacc("TRN2", target_bir_lowering=False, debug=False,
                   num_devices=N_CORES)
    xqT = nc.dram_tensor("xqT", [D, SQ], xd, kind="ExternalInput")
    xkT = nc.dram_tensor("xkT", [D, LK], xd, kind="ExternalInput")
    xvT = nc.dram_tensor("xvT", [D, LK], xd, kind="ExternalInput")
    wq = nc.dram_tensor("wq", [D, DL], xd, kind="ExternalInput")
    wk = nc.dram_tensor("wk", [D, DL], xd, kind="ExternalInput")
    wv = nc.dram_tensor("wv", [D, DL], xd, kind="ExternalInput")
    wo = nc.dram_tensor("wo", [DL, D], md, kind="ExternalInput")
    mask = nc.dram_tensor("mask", [128, KT], f32, kind="ExternalInput")
    out = nc.dram_tensor("out", [SQ, D], od, kind="ExternalOutput")
    dbg = os.environ.get("BASS_MHA_DEBUG") == "1"
    if dbg:
        dbg_qt = nc.dram_tensor("dbg_qt", [128, 2, SQ], md, kind="ExternalOutput")
        dbg_kt = nc.dram_tensor("dbg_kt", [128, 2, LK], md, kind="ExternalOutput")
        dbg_v = nc.dram_tensor("dbg_v", [128, KT, 4, 128], md, kind="ExternalOutput")

    with tile.TileContext(nc) as tc:
        with tc.tile_pool(name="singles", bufs=1) as sg:
            wq_sb = sg.tile([128, KD, DL], xd)
            wk_sb = sg.tile([128, KD, DL], xd)
            wv_sb = sg.tile([128, KD, DL], xd)
            wo_sb = sg.tile([128, DL // 128, D], md)
            mask_sb = sg.tile([128, KT], f32)
            kt_sb = sg.tile([128, 2, LK], md)       # K^T  [d_local, key]
            v_sb = sg.tile([128, KT, 4, 128], md)   # V''  [key, head, d | ones]
            qt_sb = sg.tile([128, 2, SQ], md)       # Q^T  [d_local, query]
            ctxT_sb = sg.tile([128, 2, SQ], md)     # Ctx^T normalized

            # DMA issue order = arrival order: K path, Q path, V path, Wo
            nc.sync.dma_start(out=mask_sb, in_=mask[:, :])
            nc.sync.dma_start(out=wk_sb, in_=wk[:, :].rearrange("(k p) j -> p k j", p=128))
            if md == f16:
                nc.vector.memset(v_sb, 1.0)
            else:
                nc.vector.memset(v_sb.bitcast(f32), 1.0)
            # dummy exp: pulls the ~2.7us activation-table load off phase C's
            # critical path (ACT is otherwise idle until the first softmax)
            warm_sb = sg.tile([1, 1], f32)
            nc.scalar.activation(warm_sb, mask_sb[0:1, 0:1],
                                 mybir.ActivationFunctionType.Exp)

            # ---- resident input streams (DMA priority: xk, xq, xv) ----
            strm_cm = tc.tile_pool(name="streams", bufs=1)
            strm = strm_cm.__enter__()
            xk_full = strm.tile([128, KD, LK], xd)
            xq_full = strm.tile([128, KD, SQ], xd)
            xv_full = strm.tile([128, KD, LK], xd)
            for k in range(KD):
                nc.sync.dma_start(out=xk_full[:, k, :],
                                  in_=xkT[k * 128:(k + 1) * 128, :])

            # ---- Phase A1: K^T = (Wk^T blocks) @ Xk^T, [256, LK] ----
            with tc.tile_pool(name="psA", bufs=1, space="PSUM") as psA:
                accs = {}
                for m in range(2):
                    for ci, (c0, cw) in enumerate(kchunks):
                        accs[(m, ci)] = psA.tile([128, cw], f32,
                                                 tag=f"kt{m}_{ci}", name=f"kt{m}_{ci}")
                for k in range(KD):
                    for m in range(2):
                        for ci, (c0, cw) in enumerate(kchunks):
                            nc.tensor.matmul(accs[(m, ci)],
                                             wk_sb[:, k, m * 128:(m + 1) * 128],
                                             xk_full[:, k, c0:c0 + cw],
                                             start=(k == 0), stop=(k == KD - 1))
                for m in range(2):
                    for ci, (c0, cw) in enumerate(kchunks):
                        nc.vector.tensor_copy(kt_sb[:, m, c0:c0 + cw], accs[(m, ci)])

            # ---- Phases B + A2: Q^T and V projections riding the DMA ----
            # xq streams in right after xk; Q accumulates in narrow passes
            # (2 or 4 PSUM banks) so the V accumulators (1 bank per key
            # tile, bank-aligned: matmul start=True clears a whole bank)
            # fit alongside. Pass 0 rides the xq stream; V rides xv.
            nc.sync.dma_start(out=wq_sb, in_=wq[:, :].rearrange("(k p) j -> p k j", p=128))
            if KT <= 8:
                qpass, nchunk = 2, 2
            else:
                qpass, nchunk = 4, 1
            maxg = 8 - 2 * nchunk
            vgroups = [list(range(g0, min(g0 + maxg, KT)))
                       for g0 in range(0, KT, maxg)]
            with tc.tile_pool(name="psB", bufs=1, space="PSUM") as psB, \
                 tc.tile_pool(name="psV", bufs=1, space="PSUM") as psV:
                for k in range(KD):
                    nc.sync.dma_start(out=xq_full[:, k, :],
                                      in_=xqT[k * 128:(k + 1) * 128, :])
                nc.sync.dma_start(out=wv_sb, in_=wv[:, :].rearrange("(k p) j -> p k j", p=128))
                for k in range(KD):
                    nc.sync.dma_start(out=xv_full[:, k, :],
                                      in_=xvT[k * 128:(k + 1) * 128, :])
                for p in range(qpass):
                    chunks = list(range(p * nchunk, (p + 1) * nchunk))
                    qaccs = {}
                    for m in range(2):
                        for c in chunks:
                            qaccs[(m, c)] = psB.tile([128, 512], f32,
                                                     tag=f"q{m}_{c % nchunk}",
                                                     name=f"qp{p}_{m}_{c}")
                    for k in range(KD):
                        for m in range(2):
                            for c in chunks:
                                nc.tensor.matmul(
                                    qaccs[(m, c)],
                                    wq_sb[:, k, m * 128:(m + 1) * 128],
                                    xq_full[:, k, c * 512:(c + 1) * 512],
                                    start=(k == 0), stop=(k == KD - 1))
                    if p < len(vgroups):
                        g = vgroups[p]
                        vacc = psV.tile([128, len(g), 512], f32, tag="vg",
                                        name=f"vg{p}")
                        for k in range(KD):
                            for vi, t in enumerate(g):
                                nc.tensor.matmul(
                                    vacc[:, vi, 0:DL],
                                    xv_full[:, k, t * 128:(t + 1) * 128],
                                    wv_sb[:, k, :],
                                    start=(k == 0), stop=(k == KD - 1),
                                    skip_group_check=True)
                        for vi, t in enumerate(g):
                            for hh in range(4):
                                nc.vector.tensor_copy(
                                    v_sb[:, t, hh, 0:64],
                                    vacc[:, vi, hh * 64:(hh + 1) * 64])
                    for m in range(2):
                        for c in chunks:
                            nc.vector.tensor_copy(
                                qt_sb[:, m, c * 512:(c + 1) * 512], qaccs[(m, c)])

            nc.sync.dma_start(out=wo_sb, in_=wo[:, :].rearrange("(k p) j -> p k j", p=128))

            # ---- Phase C: per-head attention ----
            # scores^T tile = K^T_h.T @ Q^T_h  -> exp(bias=mask) -> P^T
            # ctx'' = V''.T @ P^T : rows 0-63 ctx, rows 64-127 denominator
            with tc.tile_pool(name="pt", bufs=8) as ptp, \
                 tc.tile_pool(name="misc", bufs=4) as mp, \
                 tc.tile_pool(name="psC", bufs=2, space="PSUM") as psC, \
                 tc.tile_pool(name="psS", bufs=2, space="PSUM") as psS:
                for half in range(2):
                    for hh in range(4):
                        mt, mo = hh // 2, 64 * (hh % 2)
                        h0 = half * 1024
                        ctx_ps = psC.tile([128, 1024], f32, tag="ctx")
                        for t in range(KT):
                            pt_t = ptp.tile([128, 1024], md, tag="pt")
                            s_ps = psS.tile([128, 1024], f32, tag="s")
                            for cq in range(2):
                                nc.tensor.matmul(
                                    s_ps[:, cq * 512:(cq + 1) * 512],
                                    kt_sb[mo:mo + 64, mt, t * 128:(t + 1) * 128],
                                    qt_sb[mo:mo + 64, mt, h0 + cq * 512:h0 + (cq + 1) * 512],
                                    start=True, stop=True)
                            nc.scalar.activation(
                                pt_t, s_ps,
                                mybir.ActivationFunctionType.Exp,
                                bias=mask_sb[:, t:t + 1], scale=0.125)
                            for c in range(2):
                                nc.tensor.matmul(ctx_ps[:, c * 512:(c + 1) * 512],
                                                 v_sb[:, t, hh, :],
                                                 pt_t[:, c * 512:(c + 1) * 512],
                                                 start=(t == 0), stop=(t == KT - 1),
                                                 skip_group_check=True)
                        # rows 64-127 of ctx_ps all hold the softmax denominator
                        rcb = mp.tile([64, 1024], f32, tag="rcb")
                        nc.vector.reciprocal(rcb, ctx_ps[64:128, :])
                        nc.vector.tensor_mul(ctxT_sb[mo:mo + 64, mt, h0:h0 + 1024],
                                             ctx_ps[0:64, :], rcb)

            # ---- Phase D: partial output projection ----
            with tc.tile_pool(name="po", bufs=6) as pop, \
                 tc.tile_pool(name="psD", bufs=4, space="PSUM") as psD:
                for qi in range(SQ // 128):
                    o_ps = psD.tile([128, D], f32, tag="o")
                    for n in range(2):
                        for kk in range(2):
                            nc.tensor.matmul(o_ps[:, n * 512:(n + 1) * 512],
                                             ctxT_sb[:, kk, qi * 128:(qi + 1) * 128],
                                             wo_sb[:, kk, n * 512:(n + 1) * 512],
                                             start=(kk == 0), stop=(kk == 1))
                    o_sb = pop.tile([128, D], od, tag="o_sb")
                    nc.scalar.copy(o_sb[:, 0:512], o_ps[:, 0:512])
                    nc.vector.tensor_copy(o_sb[:, 512:1024], o_ps[:, 512:1024])
                    nc.sync.dma_start(out=out[qi * 128:(qi + 1) * 128, :], in_=o_sb)
            strm_cm.__exit__(None, None, None)
            if dbg:
                nc.sync.dma_start(out=dbg_qt[:, :, :], in_=qt_sb)
                nc.sync.dma_start(out=dbg_kt[:, :, :], in_=kt_sb)
                nc.sync.dma_start(out=dbg_v[:, :, :, :], in_=v_sb)
    nc.compile()
    return nc


def kernel(**inputs):
    global last_results, last_exec_wall_s
    from concourse.bass_utils import run_bass_kernel_spmd

    # BASS_TRACE needs the axon NTFF hook; disable tracing when the hook
    # module is unavailable so a stray env var cannot crash the run.
    if os.environ.get("BASS_TRACE"):
        try:
            from antenv import axon_hooks  # noqa: F401
        except Exception:
            os.environ["BASS_NEVER_TRACE"] = "1"

    q = np.asarray(inputs["queries"], dtype=np.float32)
    kx = np.asarray(inputs["keys"], dtype=np.float32)
    vx = np.asarray(inputs["values"], dtype=np.float32)
    vl = np.asarray(inputs["valid_lens"], dtype=np.int64).reshape(B)
    Wq = np.asarray(inputs["Wq"], dtype=np.float32)
    Wk = np.asarray(inputs["Wk"], dtype=np.float32)
    Wv = np.asarray(inputs["Wv"], dtype=np.float32)
    Wo = np.asarray(inputs["Wo"], dtype=np.float32)
    assert q.shape == (B, SQ, D) and kx.shape == (B, SK, D) and vx.shape == (B, SK, D)

    lens = np.clip(vl, 1, SK)
    lmax = int(lens.max())
    KT = (lmax + 127) // 128
    LK = KT * 128

    key = (KT, PREC)
    if key not in _NC_CACHE:
        _NC_CACHE[key] = _build(KT)
    nc = _NC_CACHE[key]

    xdt = np.float16 if PREC == "f16" else np.float32

    in_maps = []
    for c in range(N_CORES):
        b, hg = c // 4, c % 4
        cols = slice(DL * hg, DL * (hg + 1))
        m = np.where(np.arange(LK) < lens[b], 0.0, NEG).astype(np.float32)
        in_maps.append({
            "xqT": np.ascontiguousarray(q[b].T.astype(xdt)),
            "xkT": np.ascontiguousarray(kx[b, :LK].T.astype(xdt)),
            "xvT": np.ascontiguousarray(vx[b, :LK].T.astype(xdt)),
            "wq": np.ascontiguousarray(Wq[:, cols].astype(xdt)),
            "wk": np.ascontiguousarray(Wk[:, cols].astype(xdt)),
            "wv": np.ascontiguousarray(Wv[:, cols].astype(xdt)),
            "wo": np.ascontiguousarray(Wo[cols, :].astype(xdt)),
            "mask": np.ascontiguousarray(m.reshape(KT, 128).T),
        })

    t0 = time.perf_counter()
    res = run_bass_kernel_spmd(nc, in_maps, core_ids=list(range(N_CORES)))
    last_exec_wall_s = time.perf_counter() - t0
    last_results = res

    outs = [res.results[c]["out"].astype(np.float32) for c in range(N_CORES)]
    full = np.stack([outs[0] + outs[1] + outs[2] + outs[3],
                     outs[4] + outs[5] + outs[6] + outs[7]])
    return full.astype(np.float32)
